# revision 1
# baseline (speedup 1.0000x reference)
"""Trainium2 Bass kernel for nn_Attention_12034498363898.

Per batch b (B=8 batches, one NeuronCore each, no collectives):
  xs = x[::2,::2,::2]                     (4096, 64)
  f = xs@Wf+bf; g = xs@Wg+bg              (4096, 8) each
  s = g @ f^T (4096,4096); e = exp(s)
  hv = xs_aug @ Whv_aug                   (4096, 65)  [fused h@Wv*gamma + picker]
  v_unnorm[q] = sum_m e[m,q] * hv[m]      (4096, 65)  col 64 = sumexp
  v = v_unnorm[:, :64] / sumexp           == gamma*(softmax(s)@h@Wv + bv)
  out = x + Up2x(v)

Key structure (all chosen against the TimelineSim cost model):
  - s^T computed chunk-wise [128 keys, 512 queries] via fp8e4 DoubleRow
    matmuls (0.5 cycles/col): operands stored [8, (2, N)] with the second
    k-tile half zeroed, so no partition regrouping is needed.
  - exp split between ACT (exact) and DVE (Schraudolph bf16-bit trick);
    this PSUM->SBUF crossing is the bottleneck (GPSIMD has no PSUM port).
  - v accumulated with e^T chunks as the *stationary* operand and hv as
    the moving operand: only 65 columns streamed per [128q x 65] psum
    accumulation, queries land on partitions (natural layout, no
    transposes downstream).
  - Wh/bh/Wv/bv/gamma all folded host-side into Whv_aug [65, 65]
    (col 64 picks the xs ones-row => sumexp column).
  - Normalization: one reciprocal [128,4] + one broadcast-multiply per
    512-query block, output bf16.
  - Residual in 4 double-groups of [128, 64-row] blocks: x/out move in 4
    contiguous 2 MiB DMAs each; v bounced through a DRAM scratch (bf16)
    and gathered back with plain contiguous-partition DMAs so Up2x
    becomes a free-dim broadcast; adds on GPSIMD (SBUF-only engine,
    otherwise idle), split per l5-half (ISA allows 3 free AP dims).
  - Software pipelined: iteration i runs s/exp for query block i and the
    v-matmuls for block i-1; each residual group chases its second block.
  - Walrus accepts only ONE sem-wait per instruction; extra waits ride on
    same-engine Drain carriers (the _MAX_WAITS machinery below), so rings
    are sized/ordered to keep those waits pre-satisfied.
"""

import numpy as np

import concourse.bass as bass
import concourse.mybir as mybir
import concourse.tile as tile
from concourse.bass_utils import run_bass_kernel_spmd
from concourse.vector_clock import ScopedClock

# ---------------------------------------------------------------------------
# Workaround: this neuronxcc/walrus build rejects instructions with more than
# one sync-wait command ("Too many sync wait commands" in setupSyncWait).
# (a) TileContext's exit drain carries every outstanding wait -> split into a
#     chain of 1-wait drains.
# (b) Body instructions can get multiple waits from the scheduler -> move
#     extras onto Drain carriers inserted just before, same engine.
_MAX_WAITS = 1


def _split_drain_and_barrier(self, tick_clock, wait_clock):
    import bass_rust

    drain_inst = self.nc.sync.drain()
    wait_clock.add_sem_waits(
        drain_inst.ins, ScopedClock({None: tick_clock.global_clock})
    )
    si = drain_inst.ins.sync_info
    waits = list(si.on_wait)
    if len(waits) > _MAX_WAITS:
        si.on_wait = waits[:_MAX_WAITS]
        drain_inst.ins.sync_info = si
        for k in range(_MAX_WAITS, len(waits), _MAX_WAITS):
            extra = self.nc.sync.drain()
            esi = extra.ins.sync_info
            if esi is None:
                esi = bass_rust.SyncInfo(
                    on_wait=waits[k : k + _MAX_WAITS], on_update=[]
                )
            else:
                esi.on_wait = waits[k : k + _MAX_WAITS]
            extra.ins.sync_info = esi

    self.nc.all_engine_barrier()
    assert self.sems is not None
    popped = self.nc._tile_sem_poison_stack.pop()
    assert popped is self._sem_poison
    self.nc.clear_and_free_semaphores(list(self.sems.allocated().values()))
    self.nc.all_engine_barrier()


tile.TileContext._drain_and_barrier = _split_drain_and_barrier

_orig_lower_ordered = tile.TileContext._lower_ordered_insts


def _split_waits_lower(self, ordered):
    import bass_rust

    for bb, insts in ordered.items():
        new = []
        for inst in insts:
            si = getattr(inst, "sync_info", None)
            waits = list(si.on_wait) if si is not None else []
            if len(waits) > _MAX_WAITS:
                eng = inst.engine
                for w in waits[:-_MAX_WAITS]:
                    carrier = self.nc.engines[eng].drain(fusable=False).ins
                    csi = carrier.sync_info
                    if csi is None:
                        csi = bass_rust.SyncInfo(on_wait=[w], on_update=[])
                    else:
                        csi.on_wait = [w]
                    carrier.sync_info = csi
                    new.append(carrier)
                si.on_wait = waits[-_MAX_WAITS:]
                inst.sync_info = si
            new.append(inst)
        insts[:] = new
    return _orig_lower_ordered(self, ordered)


tile.TileContext._lower_ordered_insts = _split_waits_lower
# ---------------------------------------------------------------------------

F32 = mybir.dt.float32
I16 = mybir.dt.int16
BF16 = mybir.dt.bfloat16
FP8 = mybir.dt.float8e4

B = 8
HH = 32
N = 4096          # subsampled positions per batch
C = 64
NROWS = 32768     # full-res rows per batch
NB = 8            # query blocks of 512
MC = 32           # key chunks of 128

# Schraudolph fast-exp (bf16-bits variant): exp(x) ~= bf16_bits(x*A + Bc)
SCH_A = 184.6650
SCH_B = 16248.58

# exp engine pattern per sT tile (16 tiles of [128,1024] per block):
# 'A' = ACT exact exp, 'D' = DVE Schraudolph
EXP_PATTERN = "AADAADAADADADADA"


def build_kernel(exp_pattern=EXP_PATTERN, phases=99):
    nc = bass.Bass()

    x = nc.declare_dram_parameter("x", [NROWS, C], F32, isOutput=False)
    wfg = nc.declare_dram_parameter("wfg", [65, 16], BF16, isOutput=False)
    whv = nc.declare_dram_parameter("whv", [65, 65], BF16, isOutput=False)
    ident = nc.declare_dram_parameter("ident", [128, 128], F32, isOutput=False)
    zeros8 = nc.declare_dram_parameter("zeros8", [8, 8192], FP8, isOutput=False)
    onesbf = nc.declare_dram_parameter("onesbf", [1, N], BF16, isOutput=False)
    out = nc.declare_dram_parameter("out", [NROWS, C], F32, isOutput=True)

    vscratch = nc.dram_tensor("vscratch", [N, C], BF16)

    # xs chunk DMA view: chunk mc holds xs rows [128*mc, 128*mc+128), i.e.
    # DRAM row 4096*m0 + 2048*jh + 512*jw + 64*w2h + 2*d2 for mc=4*m0+(jh,jw),
    # partition p = (w2h, d2).
    # [16 h2, 16 w2, 16 d2, 64]: subsampled rows; chunk mc = h2*2 + (w2>=8)
    x_sub = x.rearrange(
        "(h2 hb w2 wb d2 db) c -> hb wb db h2 w2 d2 c",
        h2=16, hb=2, w2=16, wb=2, d2=16, db=2,
    )[0, 0, 0]

    # residual double-group view: group G covers rows [G*8192, (G+1)*8192),
    # partition p <- 64 consecutive rows. With this blocking the only
    # partition-duplicated v bit sits in contiguous 16-partition runs, so
    # every vw gather DMA is a plain contiguous copy.
    x_blk = x.rearrange("(G p l) c -> G p (l c)", G=4, p=128)
    out_blk = out.rearrange("(G p l) c -> G p (l c)", G=4, p=128)

    # v slabs: s = 4G + (p>>5) selects 256 consecutive v rows; partition
    # p65*32 + b4*16 + k holds rows [s*256 + 16k, s*256 + 16k + 16)
    vsc_r = vscratch.rearrange(
        "(s k vl) c -> s k (vl c)", s=16, k=16,
    )  # [16, 16, 1024]

    # v_norm scatter: block j writes v rows [512j, 512j+512): row = qc*128+p
    vsc_w = vscratch.rearrange("(j qc p) c -> j p qc c", j=8, qc=4)

    with tile.TileContext(nc) as tc:
        with (
            tc.tile_pool(name="const", bufs=1) as const_pool,
            tc.tile_pool(name="persist", bufs=1) as persist,
            tc.tile_pool(name="xin", bufs=5) as xin_pool,
            tc.tile_pool(name="eT", bufs=2) as eT_pool,
            tc.tile_pool(name="vn", bufs=2) as vn_pool,
            tc.tile_pool(name="vw", bufs=2) as vw_pool,
            tc.tile_pool(name="oout", bufs=1) as oout_pool,
            tc.tile_pool(name="stage", bufs=4) as stage_pool,
        ):
            # ---- constants (ident/ones on SP ahead of xs; rest on ACT) ----
            id_sb = const_pool.tile([128, 128], F32)
            nc.sync.dma_start(id_sb[:], ident[:])
            xsT = persist.tile([65, N], BF16)
            nc.sync.dma_start(xsT[64:65, :], onesbf[:])
            wfg_sb = const_pool.tile([65, 16], BF16)
            nc.scalar.dma_start(wfg_sb[:], wfg[:])
            whv_sb = const_pool.tile([65, 65], BF16)
            nc.scalar.dma_start(whv_sb[:], whv[:])
            # fg_dr: [8 ch-partitions, (f/g, k-tile t, n)] fp8; t=1 zeroed
            fg_dr = persist.tile([8, 2, 2, N], FP8)
            nc.scalar.dma_start(
                fg_dr[:, :, 1, :],
                zeros8[:].rearrange("p (f n) -> p f n", f=2),
            )
            hv_sb = persist.tile([128, MC, 65], BF16)

            # residual x tiles: loaded lazily (two ahead of the residual
            # consumer) so their transfers don't crowd out phase P's xs loads
            x_t = {}

            def load_x(g):
                xt = xin_pool.tile([128, 4096], F32, tag="xin", name=f"xt{g}")
                x_t[g] = xt
                nc.sync.dma_start(xt[:], x_blk[g])

            # ---- fused projections + attention ----
            # All PSUM-producing projection work allocates slices of the same
            # rotating sT pool, and block 0's s/exp interleaves between
            # projection groups, so the pipeline fills immediately.
            with (
                tc.tile_pool(name="sT", bufs=3, space=bass.MemorySpace.PSUM) as sT_pool,
                tc.tile_pool(name="wrk", bufs=2, space=bass.MemorySpace.PSUM) as wrk_pool,
            ):
                def proj_group(g):
                    st = stage_pool.tile([128, 4, C], F32, tag="xs_st")
                    for j in range(4):
                        mc = 4 * g + j
                        q = nc.sync if mc % 2 == 0 else nc.scalar
                        q.dma_start(
                            st[:, j, :],
                            x_sub[mc >> 1, 8 * (mc & 1) : 8 * (mc & 1) + 8],
                        )
                    # transposes share one psum bank (start only for the
                    # first; the rest write into the pending-zeroed bank)
                    w = wrk_pool.tile([128, 512], F32, tag="wrk",
                                      name=f"proj{g}")
                    pt = w[0:64, :].rearrange("p (a b) -> p a b", a=4)
                    for j in range(4):
                        nc.tensor.matmul(
                            pt[:, j, :], st[:, j, :], id_sb[:],
                            start=(j == 0), stop=(j == 3), is_transpose=True,
                        )
                    nc.scalar.copy(
                        xsT[0:64, g * 512 : (g + 1) * 512], w[0:64, :]
                    )
                    wf = wrk_pool.tile([128, 512], F32, tag="wrk",
                                       name=f"pf{g}")
                    nc.tensor.matmul(
                        wf[0:8, :], wfg_sb[:, 0:8],
                        xsT[:, g * 512 : (g + 1) * 512],
                        start=True, stop=True,
                    )
                    nc.scalar.copy(
                        fg_dr[:, 0, 0, g * 512 : (g + 1) * 512], wf[0:8, :]
                    )
                    wg = wrk_pool.tile([128, 512], F32, tag="wrk",
                                       name=f"pg{g}")
                    nc.tensor.matmul(
                        wg[0:8, :], wfg_sb[:, 8:16],
                        xsT[:, g * 512 : (g + 1) * 512],
                        start=True, stop=True,
                    )
                    nc.vector.tensor_copy(
                        fg_dr[:, 1, 0, g * 512 : (g + 1) * 512], wg[0:8, :]
                    )

                def hv_group(g):
                    w = wrk_pool.tile([128, 512], F32, tag="wrk",
                                      name=f"phv{g}")
                    phv = w[:, 0:260].rearrange("p (a b) -> p a b", a=4)
                    for j in range(4):
                        mc = 4 * g + j
                        nc.tensor.matmul(
                            phv[:, j, :],
                            xsT[:, mc * 128 : (mc + 1) * 128],
                            whv_sb[:],
                            start=(j == 0), stop=(j == 3),
                        )
                    if g % 2 == 0:
                        nc.vector.tensor_copy(
                            hv_sb[:, 4 * g : 4 * g + 4, :], phv[:]
                        )
                    else:
                        nc.scalar.copy(
                            hv_sb[:, 4 * g : 4 * g + 4, :], phv[:]
                        )

                eT_prev = None
                pending_out = []
                for i in range(NB + 1):
                    e_cur = None
                    if i < NB:
                        e_cur = eT_pool.tile(
                            [128, MC, 512], BF16, tag="eT", name=f"eT{i}"
                        )
                    # all 4 qc accumulation regions share ONE psum bank: a
                    # single accumulation group spanning all 128 v-matmuls
                    vps = inv = v_nrm = None
                    if i > 0:
                        vps_w = wrk_pool.tile([128, 512], F32, tag="wrk",
                                              name=f"vps{i}")
                        vps = vps_w[:, 0:260].rearrange(
                            "p (a b) -> p a b", a=4
                        )
                        inv = vn_pool.tile([128, 4], F32, tag="inv")
                        v_nrm = vn_pool.tile([128, 4, C], BF16, tag="vnrm")

                    def v_matmuls(vps, e_cur, mc_list):
                        for mc in mc_list:
                            for qc in range(4):
                                nc.tensor.matmul(
                                    vps[:, qc, :],
                                    e_cur[:, mc, qc * 128 : (qc + 1) * 128],
                                    hv_sb[:, mc, :],
                                    start=(mc == 0 and qc == 0),
                                    stop=(mc == MC - 1 and qc == 3),
                                )

                    if i == 0:
                        for g in range(4):
                            proj_group(g)
                    for t in range(16):
                        if i == 0 and t in (0, 2, 4, 6):
                            proj_group(t // 2 + 4)
                        if i < NB:
                            sT = sT_pool.tile([128, 2, 512], F32, tag="sT")
                            for k in range(2):
                                mc = 2 * t + k
                                nc.tensor.matmul(
                                    sT[:, k, :],
                                    fg_dr[:, 0, :, mc * 128 : (mc + 1) * 128],
                                    fg_dr[:, 1, :, i * 512 : (i + 1) * 512],
                                    start=True, stop=True,
                                    perf_mode=mybir.MatmulPerfMode.DoubleRow,
                                )
                            dst = e_cur[:, 2 * t : 2 * t + 2, :]
                            if exp_pattern[t] == "A":
                                nc.scalar.activation(
                                    dst, sT[:],
                                    mybir.ActivationFunctionType.Exp,
                                )
                            else:
                                nc.vector.tensor_scalar(
                                    dst.bitcast(I16), sT[:], SCH_A, SCH_B,
                                    mybir.AluOpType.mult, mybir.AluOpType.add,
                                )
                        if i % 2 == 0 and t == 7 and i // 2 < 4:
                            load_x(i // 2)
                        if i == 0 and t == 15:
                            for g in range(8):
                                hv_group(g)
                        if i > 0 and t < 8:
                            # all of block i-1's v-accumulation in the first
                            # half of the iteration so the norm can run
                            # mid-iteration (keeps the vps ring from lagging)
                            v_matmuls(vps, eT_prev, range(4 * t, 4 * t + 4))
                        if i > 0 and t == 8:
                            nc.vector.reciprocal(
                                inv[:].rearrange("p (q c) -> p q c", c=1),
                                vps[:, :, 64:65],
                            )
                            nc.vector.tensor_tensor(
                                v_nrm[:],
                                vps[:, :, 0:64],
                                inv[:].rearrange("p (q c) -> p q c", c=1)
                                .broadcast_to([128, 4, C]),
                                mybir.AluOpType.mult,
                            )
                    eT_prev = e_cur if i < NB else eT_prev
                    j = i - 1
                    if j < 0:
                        continue
                    if pending_out:
                        jo, oto = pending_out.pop(0)
                        nc.sync.dma_start(out_blk[jo], oto[:])
                    # bounce v through DRAM scratch (SP queue: its waits
                    # must not block the ACT queue's exp stream)
                    nc.sync.dma_start(vsc_w[j], v_nrm[:])
                    if j % 2 == 1:
                        # residual for double-group G (needs v blocks 2G, 2G+1)
                        G = j >> 1
                        vw = vw_pool.tile([128, 16, C], BF16, tag="vw")
                        vw_v = vw[:].rearrange("(h k) a c -> h k (a c)", h=8)
                        for p65 in range(4):
                            for b4 in range(2):
                                nc.sync.dma_start(
                                    vw_v[p65 * 2 + b4], vsc_r[4 * G + p65]
                                )
                        ot = oout_pool.tile([128, 4096], F32, tag="oout")
                        vb = vw[:].rearrange(
                            "p (u vl) (w c) -> p u vl w c", u=1, w=1
                        ).broadcast_to([128, 2, 16, 2, C])
                        xt_v = x_t[G][:].rearrange(
                            "p (l5 vl db c) -> p l5 vl db c",
                            l5=2, vl=16, db=2,
                        )
                        ot_v = ot[:].rearrange(
                            "p (l5 vl db c) -> p l5 vl db c",
                            l5=2, vl=16, db=2,
                        )
                        # ISA allows only 3 free AP dims: one add per l5
                        # half; the last group's second half goes to DVE so
                        # the tail drains in parallel
                        nc.gpsimd.tensor_tensor(
                            ot_v[:, 0], xt_v[:, 0], vb[:, 0],
                            mybir.AluOpType.add,
                        )
                        eng2 = nc.vector if G == 3 else nc.gpsimd
                        eng2.tensor_tensor(
                            ot_v[:, 1], xt_v[:, 1], vb[:, 1],
                            mybir.AluOpType.add,
                        )
                        pending_out.append((G, ot))
                for jo, oto in pending_out:
                    nc.sync.dma_start(out_blk[jo], oto[:])

    return nc


_CACHE = {}


def _get_nc():
    if "nc" not in _CACHE:
        _CACHE["nc"] = build_kernel()
    return _CACHE["nc"]


def _make_in_maps(inputs):
    import ml_dtypes

    bf16 = ml_dtypes.bfloat16
    fp8 = ml_dtypes.float8_e4m3
    x = np.asarray(inputs["x"], dtype=np.float32)
    gamma_v = float(np.asarray(inputs["gamma"]).reshape(-1)[0])

    wfg = np.zeros((65, 16), np.float32)
    wfg[:64, 0:8] = np.asarray(inputs["Wf"])
    wfg[64, 0:8] = np.asarray(inputs["bf"])
    wfg[:64, 8:16] = np.asarray(inputs["Wg"])
    wfg[64, 8:16] = np.asarray(inputs["bg"])

    wh_aug = np.zeros((65, 33), np.float32)
    wh_aug[:64, :32] = np.asarray(inputs["Wh"])
    wh_aug[64, :32] = np.asarray(inputs["bh"])
    wh_aug[64, 32] = 1.0
    wv_aug = np.concatenate(
        [np.asarray(inputs["Wv"]), np.asarray(inputs["bv"])[None, :]], 0
    ).astype(np.float32)
    whv = np.zeros((65, 65), np.float32)
    whv[:, :64] = (wh_aug @ wv_aug) * gamma_v
    whv[64, 64] = 1.0

    shared = {
        "wfg": wfg.astype(bf16),
        "whv": whv.astype(bf16),
        "ident": np.eye(128, dtype=np.float32),
        "zeros8": np.zeros((8, 8192), np.float32).astype(fp8),
        "onesbf": np.ones((1, N), np.float32).astype(bf16),
    }
    return [
        dict(shared, x=np.ascontiguousarray(x[b].reshape(NROWS, C)))
        for b in range(B)
    ]


def kernel(x, Wf, bf, Wg, bg, Wh, bh, Wv, bv, gamma):
    nc = _get_nc()
    in_maps = _make_in_maps(dict(
        x=x, Wf=Wf, bf=bf, Wg=Wg, bg=bg, Wh=Wh, bh=bh, Wv=Wv, bv=bv,
        gamma=gamma,
    ))
    res = run_bass_kernel_spmd(nc, in_maps, list(range(B)))
    outs = [res.results[b]["out"].reshape(HH, HH, HH, C) for b in range(B)]
    return np.stack(outs).astype(np.float32)


if __name__ == "__main__":
    import reference

    inputs = {k: np.asarray(v) for k, v in reference.setup_inputs().items()}
    got = kernel(**inputs)
    exp = np.asarray(reference.reference(**inputs))
    err = np.abs(got - exp).max() / (np.abs(exp).max() + 1e-30)
    print("Relative error:", err)

